# revision 9
# baseline (speedup 1.0000x reference)
"""Trainium2 Bass kernel for nn_MultiHeadAttention_60559038873660.

Reference math (faithful to the source bug: attention is contracted with the
projected K, not V, so v/Wv are dead inputs):
    qp = q @ Wq.T ; kp = k @ Wk.T
    head split via reshape(b, l, 64, 16): head n takes strided columns {d*16+n}
    S = Qh @ Kh.T / 8 ; A = softmax(S, axis=m) ; X = A @ Kh ; out = X @ Wo.T

Strategy (v2 — PE-saturation restructure):
  - Host-side: permute weight rows/cols head-major; pre-pack q/k/weights into
    strip-major layouts so every DMA is contiguous (>=4KB lines).
  - 8 cores = 2 batches x 4 head-groups (4 heads each). Host sums the 4
    output partials per batch (tensor-parallel row-split reduction).
  - The PE has DVFS p-states: it reaches full rate only after ~3us of
    gap-free execution, and idle gaps reset it. So the whole kernel is
    scheduled to keep the PE busy from ~4us with zero idle:
      prologue: kproj g0 (4 strips, fed by 2 DMA queues) -> qproj lt0 g0
      attention g0: score/exp/X per mc, with all remaining projections
        (qproj lt1-3 g0, kproj/qproj g1) popped as fillers into PE slack
      attention g1: out-projection tiles for completed strips as fillers
      tail: last strip's out tiles
  - k^T -> khp head-major transposes run on the DMA XBAR (dma transpose),
    not the PE.
  - Softmax denominators fall out of the X^T matmul via a fused ones column;
    normalization via reciprocal + DRAM-broadcast + VectorE multiply.
"""

import contextlib
import ctypes
import os
import sys
import types

import numpy as np

import concourse.bacc as bacc
import concourse.tile as tile
from concourse import mybir
from concourse.bass import ds, ts
from concourse.bass_utils import run_bass_kernel_spmd


def _install_ntff_hook():
    """Provide antenv.axon_hooks if the image lacks it, wiring NTFF
    profiling straight into libaxon_pjrt.so (same ABI trn_boot uses)."""
    try:
        import antenv.axon_hooks  # noqa: F401
        return
    except ImportError:
        pass
    mod = types.ModuleType("antenv.axon_hooks")
    holder = [None]
    mod.set_axon_ntff_profile_hook = lambda h: holder.__setitem__(0, h)
    mod.get_axon_ntff_profile_hook = lambda: holder[0]
    sys.modules["antenv.axon_hooks"] = mod
    try:
        import antenv
        antenv.axon_hooks = mod
    except ImportError:
        pass

    so_path = "/opt/axon/libaxon_pjrt.so"
    if not os.path.exists(so_path):
        return
    lib = ctypes.CDLL(so_path)
    if not hasattr(lib, "axon_start_nrt_profile"):
        return
    lib.axon_start_nrt_profile.argtypes = [ctypes.POINTER(ctypes.c_int64), ctypes.c_size_t]
    lib.axon_start_nrt_profile.restype = ctypes.c_int64
    lib.axon_stop_nrt_profile.argtypes = [ctypes.c_char_p]
    lib.axon_stop_nrt_profile.restype = ctypes.c_int64

    @contextlib.contextmanager
    def _hook(output_dir, device_ids):
        import jax
        jax.devices()
        if device_ids:
            ids = (ctypes.c_int64 * len(device_ids))(*device_ids)
            rc = lib.axon_start_nrt_profile(ids, len(device_ids))
        else:
            rc = lib.axon_start_nrt_profile(None, 0)
        if rc != 0:
            raise RuntimeError(f"axon_start_nrt_profile rc={rc}")
        try:
            yield
        finally:
            n = lib.axon_stop_nrt_profile(str(output_dir).encode())
            print(f"profile: {n} file(s) written to {output_dir}", file=sys.stderr)

    mod.set_axon_ntff_profile_hook(_hook)


_install_ntff_hook()

f32 = mybir.dt.float32
bf16 = mybir.dt.bfloat16
Exp = mybir.ActivationFunctionType.Exp

P = 128
DIM = 1024
NH = 16
HD = 64
HPC = 4          # heads per core
CW = HPC * HD    # 256 channel columns per core
CH = HD + 1      # head channels + ones column
G = CW // P      # 2 channel groups of 128
KC = DIM // P    # 8 contraction chunks for projections
NT = 512         # matmul moving-dim tile

_cache = {}


def _build(L, M):
    LT = L // NT              # q strips
    MT = M // NT              # k strips
    MG = M // P               # m chunks for attention
    L5 = L // NT              # attention l-strips per head pair
    LC = L // P               # out-proj l chunks
    JT = DIM // NT            # out-proj j tiles

    nc = bacc.Bacc()
    # strip-major host-packed layouts: every DMA is contiguous per partition
    qTs = nc.declare_dram_parameter("qTs", [LT, P, KC, NT], bf16, isOutput=False)
    kTs = nc.declare_dram_parameter("kTs", [MT, P, KC, NT], bf16, isOutput=False)
    wq = nc.declare_dram_parameter("wq", [P, KC, CW], bf16, isOutput=False)
    wk = nc.declare_dram_parameter("wk", [P, KC, CW], bf16, isOutput=False)
    wo = nc.declare_dram_parameter("wo", [P, G, DIM], bf16, isOutput=False)
    out = nc.declare_dram_parameter("out", [L, DIM], f32, isOutput=True)
    den_dram = nc.dram_tensor("den_scratch", [HPC, L], f32)
    rden_dram = nc.dram_tensor("rden_scratch", [HPC, L], f32)

    with tile.TileContext(nc) as tc:
        with (
            tc.tile_pool(name="singles", bufs=1) as singles,
            tc.tile_pool(name="io", bufs=2) as io,
            tc.tile_pool(name="es", bufs=4) as es_pool,
            tc.tile_pool(name="opool", bufs=3) as opool,
            tc.tile_pool(name="dstp", bufs=2) as dstp,
        ):
            wq_sb = singles.tile([P, KC, CW], bf16)
            wk_sb = singles.tile([P, KC, CW], bf16)
            wo_sb = singles.tile([P, G, DIM], bf16)
            kTr = singles.tile([P, MT, KC, NT], bf16)   # resident k^T
            qTr = singles.tile([P, LT, KC, NT], bf16)   # resident q^T

            qhT = singles.tile([P, G, L], bf16)
            khT = singles.tile([P, G, M], bf16)
            # per-head stride padded to 128 elems: XBAR transpose dst must be
            # 256B-aligned; cols 0:64 = Kh^T, col 64 = ones, 65:128 dead
            khp = singles.tile([P, MG, HPC, P], bf16)
            xu = singles.tile([P, G, L], bf16)
            rdbc = singles.tile([P, G, L], f32)

            ones_sb = singles.tile([P, 1], f32)
            nc.vector.memset(ones_sb, 1.0)
            for mg in range(MG):
                nc.vector.tensor_copy(khp[:, mg, :, HD:CH],
                                      ones_sb[:, None, :].to_broadcast([P, HPC, 1]))

            # ---- DMA queue plan (2 HW queues: sync + scalar) -------------
            # sync:   kTs0, kTs1, kTs2, wo, [ktrans g0], [ktrans g1], [out]
            # scalar: wq, wk, kTs3, qTs0..3   (scalar queue is free until exp
            #         starts; all its DMA issues land in the prologue)
            nc.sync.dma_start(kTr[:, 0], kTs[0])
            nc.sync.dma_start(kTr[:, 1], kTs[1])
            nc.sync.dma_start(kTr[:, 2], kTs[2])
            nc.sync.dma_start(wo_sb, wo[:, :, :])
            nc.scalar.dma_start(wq_sb, wq[:, :, :])
            nc.scalar.dma_start(wk_sb, wk[:, :, :])
            nc.scalar.dma_start(kTr[:, 3], kTs[3])
            for lt in range(LT):
                nc.scalar.dma_start(qTr[:, lt], qTs[lt])

            with (
                tc.tile_pool(name="pw", bufs=2, space="PSUM") as pw,
                tc.tile_pool(name="psS", bufs=2, space="PSUM") as psS,
                tc.tile_pool(name="psX", bufs=2, space="PSUM") as psX,
            ):
                def _proj_a(srcr, w_sb, tt, g, st):
                    ps = pw.tile([P, NT], f32, tag="pw")
                    for kc in range(KC // 2):
                        nc.tensor.matmul(ps, lhsT=w_sb[:, kc, ts(g, P)],
                                         rhs=srcr[:, tt, kc],
                                         start=(kc == 0), stop=False)
                    st["ps"] = ps

                def _proj_b(dst, srcr, w_sb, tt, g, st):
                    ps = st["ps"]
                    for kc in range(KC // 2, KC):
                        nc.tensor.matmul(ps, lhsT=w_sb[:, kc, ts(g, P)],
                                         rhs=srcr[:, tt, kc],
                                         start=False, stop=(kc == KC - 1))
                    nc.vector.tensor_copy(dst[:, g, ts(tt, NT)], ps)

                def qproj(lt, g):
                    st = {}
                    _proj_a(qTr, wq_sb, lt, g, st)
                    _proj_b(qhT, qTr, wq_sb, lt, g, st)

                def ktrans_dma(eng, mc, g):
                    # k^T chunk -> head-major khp rows via the DMA XBAR
                    # (one transpose per head: dst must be contiguous+aligned)
                    eng.dma_start(khp[:, mc, 2 * g, 0:HD],
                                  khT[0:HD, g, ts(mc, P)], transpose=True)
                    eng.dma_start(khp[:, mc, 2 * g + 1, 0:HD],
                                  khT[HD:P, g, ts(mc, P)], transpose=True)

                # ---- prologue: kproj g0 + qproj lt0 g0, PE gap-free ------
                for mt in range(MT):
                    st = {}
                    _proj_a(kTr, wk_sb, mt, 0, st)
                    _proj_b(khT, kTr, wk_sb, mt, 0, st)
                qproj(0, 0)
                for mc in range(MG):
                    ktrans_dma(nc.sync, mc, 0)

                # ---- filler actions for attention g0 PE slack ------------
                fillers = []

                def kproj_g1(mt):
                    st = {}
                    fillers.append(lambda: _proj_a(kTr, wk_sb, mt, 1, st))
                    fillers.append(lambda: _proj_b(khT, kTr, wk_sb, mt, 1, st))

                def qproj_f(lt, g):
                    st = {}
                    fillers.append(lambda: _proj_a(qTr, wq_sb, lt, g, st))
                    fillers.append(lambda: _proj_b(qhT, qTr, wq_sb, lt, g, st))

                qproj_f(1, 0)
                kproj_g1(0)
                qproj_f(2, 0)
                kproj_g1(1)
                qproj_f(3, 0)
                kproj_g1(2)
                kproj_g1(3)
                for mc_lo in range(0, MG, 4):
                    fillers.append(
                        lambda a=mc_lo: [ktrans_dma(nc.sync, mc, 1)
                                         for mc in range(a, a + 4)])
                for lt in range(LT):
                    qproj_f(lt, 1)

                # out-projection tile (2 matmuls + copy + store)
                def out_tile(lc, jt, eng):
                    po = pw.tile([P, NT], f32, tag="pw")
                    for cc in range(G):
                        nc.tensor.matmul(po, lhsT=xu[:, cc, ts(lc, P)],
                                         rhs=wo_sb[:, cc, ts(jt, NT)],
                                         start=(cc == 0), stop=(cc == G - 1))
                    ot = opool.tile([P, NT], f32, tag="ot")
                    nc.vector.tensor_copy(ot, po)
                    eng.dma_start(out[ts(lc, P), ts(jt, NT)], ot)

                pending = []  # out-proj (lc, jt) pairs, filled per finished g1 strip

                # ---- attention ------------------------------------------
                for g in range(G):
                    hA, hB = 2 * g, 2 * g + 1
                    if g == 1:
                        while fillers:
                            fillers.pop(0)()
                    for l5 in range(L5):
                        lsl = ts(l5, NT)

                        def emit_sp(mc, g=g, lsl=lsl):
                            sps = psS.tile([P, 2 * NT], f32, tag="s")
                            nc.tensor.matmul(sps[:, 0:NT],
                                             lhsT=khT[0:HD, g, ts(mc, P)],
                                             rhs=qhT[0:HD, g, lsl],
                                             start=True, stop=True)
                            nc.tensor.matmul(sps[:, NT:2 * NT],
                                             lhsT=khT[HD:P, g, ts(mc, P)],
                                             rhs=qhT[HD:P, g, lsl],
                                             start=True, stop=True)
                            return sps

                        xpsA = psX.tile([CH, NT], f32, tag="x")
                        xpsB = psX.tile([CH, NT], f32, tag="x")
                        sq = [emit_sp(0)]
                        if MG > 1:
                            sq.append(emit_sp(1))
                        for mc in range(MG):
                            if mc + 2 < MG:
                                sq.append(emit_sp(mc + 2))
                            es = es_pool.tile([P, 2 * NT], bf16, tag="es")
                            nc.scalar.activation(es, sq.pop(0), Exp, scale=0.125)
                            nc.tensor.matmul(xpsA, lhsT=khp[:, mc, hA, 0:CH],
                                             rhs=es[:, 0:NT],
                                             start=(mc == 0), stop=(mc == MG - 1))
                            nc.tensor.matmul(xpsB, lhsT=khp[:, mc, hB, 0:CH],
                                             rhs=es[:, NT:2 * NT],
                                             start=(mc == 0), stop=(mc == MG - 1))
                            if mc % 2 == 1:
                                if g == 0 and fillers:
                                    fillers.pop(0)()
                                elif g == 1 and pending:
                                    lc, jt = pending.pop(0)
                                    out_tile(lc, jt, nc.sync)

                        for hh, xps in ((0, xpsA), (1, xpsB)):
                            h = 2 * g + hh
                            pb = hh * HD
                            nc.vector.tensor_copy(xu[pb:pb + HD, g, lsl], xps[0:HD])
                            dstg = dstp.tile([1, NT], f32, tag="dst")
                            nc.vector.tensor_copy(dstg, xps[HD:CH])
                            nc.gpsimd.dma_start(den_dram[h:h + 1, lsl], dstg)
                            dsp_t = io.tile([P, NT // P], f32, tag="dsp")
                            nc.gpsimd.dma_start(
                                dsp_t, den_dram[h, lsl].rearrange("(p f) -> p f", p=P))
                            nc.vector.reciprocal(dsp_t, dsp_t)
                            nc.gpsimd.dma_start(
                                rden_dram[h, lsl].rearrange("(p f) -> p f", p=P), dsp_t)
                            nc.gpsimd.dma_start(
                                rdbc[ts(hh, HD), g, lsl],
                                rden_dram[h:h + 1, lsl].to_broadcast([HD, NT]))
                            nc.vector.tensor_mul(xu[pb:pb + HD, g, lsl],
                                                 xu[pb:pb + HD, g, lsl],
                                                 rdbc[ts(hh, HD), g, lsl])

                        if g == 1:
                            # out-proj for this strip, popped during the next
                            for lci in range(NT // P):
                                for jt in range(JT):
                                    pending.append((l5 * (NT // P) + lci, jt))

                # tail: drain remaining out tiles, alternating DMA queues
                # (the scalar queue is free once the last exp has issued)
                for i, (lc, jt) in enumerate(pending):
                    out_tile(lc, jt, nc.sync if i % 2 == 0 else nc.scalar)

    nc.finalize()
    return nc


def _get_nc(L, M):
    key = (L, M)
    if key not in _cache:
        _cache[key] = _build(L, M)
    return _cache[key]


# head-major channel permutation: new channel c = h*64+d <- original column d*16+h
_PERM = np.array([(c % HD) * NH + c // HD for c in range(DIM)])

last_exec_time_ns = None
last_results = None


def kernel(q, k, v, Wq, Wk, Wv, Wo):  # noqa: ARG001 - v/Wv dead in reference
    global last_exec_time_ns, last_results
    q = np.asarray(q, np.float32)
    k = np.asarray(k, np.float32)
    Wq = np.asarray(Wq, np.float32)
    Wk = np.asarray(Wk, np.float32)
    Wo = np.asarray(Wo, np.float32)
    B, L, _ = q.shape
    M = k.shape[1]
    LT, MT = L // NT, M // NT

    import ml_dtypes
    bf = ml_dtypes.bfloat16
    Wq_p = Wq[_PERM]            # (1024, 1024) head-major rows
    Wk_p = Wk[_PERM]
    WoT_p = Wo[:, _PERM].T      # (1024 c, 1024 j)

    def pack_strips(x):         # x: (S, DIM) -> [S/NT, P, KC, NT]
        S = x.shape[0]
        xt = x.T.reshape(KC, P, S // NT, NT)
        return np.ascontiguousarray(xt.transpose(2, 1, 0, 3)).astype(bf)

    def pack_w(Whg):            # (CW, DIM) -> [P, KC, CW]
        wt = Whg.T.reshape(KC, P, CW)
        return np.ascontiguousarray(wt.transpose(1, 0, 2)).astype(bf)

    def pack_wo(WoThg):         # (CW, DIM) -> [P, G, DIM]
        wt = WoThg.reshape(G, P, DIM)
        return np.ascontiguousarray(wt.transpose(1, 0, 2)).astype(bf)

    qTs = [pack_strips(q[b]) for b in range(B)]
    kTs = [pack_strips(k[b]) for b in range(B)]
    wq_l = [pack_w(Wq_p[hg * CW:(hg + 1) * CW, :]) for hg in range(4)]
    wk_l = [pack_w(Wk_p[hg * CW:(hg + 1) * CW, :]) for hg in range(4)]
    wo_l = [pack_wo(WoT_p[hg * CW:(hg + 1) * CW, :]) for hg in range(4)]

    in_maps = []
    for core in range(8):
        b, hg = divmod(core, 4)
        in_maps.append({"qTs": qTs[b], "kTs": kTs[b], "wq": wq_l[hg],
                        "wk": wk_l[hg], "wo": wo_l[hg]})

    nc = _get_nc(L, M)
    trace = bool(int(os.environ.get("MHA_TRACE", "0")))
    res = run_bass_kernel_spmd(nc, in_maps, core_ids=list(range(8)), trace=trace)
    last_results = res
    last_exec_time_ns = res.exec_time_ns

    out = np.zeros((B, L, DIM), np.float32)
    for core in range(8):
        b = core // 4
        out[b] += res.results[core]["out"]
    return out


# revision 13
# speedup vs baseline: 1.2550x; 1.2550x over previous
"""Trainium2 Bass kernel for nn_MultiHeadAttention_60559038873660.

Reference math (faithful to the source bug: attention is contracted with the
projected K, not V, so v/Wv are dead inputs):
    qp = q @ Wq.T ; kp = k @ Wk.T
    head split via reshape(b, l, 64, 16): head n takes strided columns {d*16+n}
    S = Qh @ Kh.T / 8 ; A = softmax(S, axis=m) ; X = A @ Kh ; out = X @ Wo.T

Strategy (v2 — PE-saturation restructure):
  - Host-side: permute weight rows/cols head-major; pre-pack q/k/weights into
    strip-major layouts so every DMA is contiguous (>=4KB lines).
  - 8 cores = 2 batches x 4 head-groups (4 heads each). Host sums the 4
    output partials per batch (tensor-parallel row-split reduction).
  - The PE has DVFS p-states: it reaches full rate only after ~3us of
    gap-free execution, and idle gaps reset it. So the whole kernel is
    scheduled to keep the PE busy from ~4us with zero idle:
      prologue: kproj g0 (4 strips, fed by 2 DMA queues) -> qproj lt0 g0
      attention g0: score/exp/X per mc, with all remaining projections
        (qproj lt1-3 g0, kproj/qproj g1) popped as fillers into PE slack
      attention g1: out-projection tiles for completed strips as fillers
      tail: last strip's out tiles
  - k^T -> khp head-major transposes run on the DMA XBAR (dma transpose),
    not the PE.
  - Softmax denominators fall out of the X^T matmul via a fused ones column;
    normalization via reciprocal + DRAM-broadcast + VectorE multiply.
"""

import contextlib
import ctypes
import os
import sys
import types

import numpy as np

import concourse.bacc as bacc
import concourse.tile as tile
from concourse import mybir
from concourse.bass import ds, ts
from concourse.bass_utils import run_bass_kernel_spmd


def _install_ntff_hook():
    """Provide antenv.axon_hooks if the image lacks it, wiring NTFF
    profiling straight into libaxon_pjrt.so (same ABI trn_boot uses)."""
    try:
        import antenv.axon_hooks  # noqa: F401
        return
    except ImportError:
        pass
    mod = types.ModuleType("antenv.axon_hooks")
    holder = [None]
    mod.set_axon_ntff_profile_hook = lambda h: holder.__setitem__(0, h)
    mod.get_axon_ntff_profile_hook = lambda: holder[0]
    sys.modules["antenv.axon_hooks"] = mod
    try:
        import antenv
        antenv.axon_hooks = mod
    except ImportError:
        pass

    so_path = "/opt/axon/libaxon_pjrt.so"
    if not os.path.exists(so_path):
        return
    lib = ctypes.CDLL(so_path)
    if not hasattr(lib, "axon_start_nrt_profile"):
        return
    lib.axon_start_nrt_profile.argtypes = [ctypes.POINTER(ctypes.c_int64), ctypes.c_size_t]
    lib.axon_start_nrt_profile.restype = ctypes.c_int64
    lib.axon_stop_nrt_profile.argtypes = [ctypes.c_char_p]
    lib.axon_stop_nrt_profile.restype = ctypes.c_int64

    @contextlib.contextmanager
    def _hook(output_dir, device_ids):
        import jax
        jax.devices()
        if device_ids:
            ids = (ctypes.c_int64 * len(device_ids))(*device_ids)
            rc = lib.axon_start_nrt_profile(ids, len(device_ids))
        else:
            rc = lib.axon_start_nrt_profile(None, 0)
        if rc != 0:
            raise RuntimeError(f"axon_start_nrt_profile rc={rc}")
        try:
            yield
        finally:
            n = lib.axon_stop_nrt_profile(str(output_dir).encode())
            print(f"profile: {n} file(s) written to {output_dir}", file=sys.stderr)

    mod.set_axon_ntff_profile_hook(_hook)


_install_ntff_hook()

f32 = mybir.dt.float32
bf16 = mybir.dt.bfloat16
Exp = mybir.ActivationFunctionType.Exp

P = 128
DIM = 1024
NH = 16
HD = 64
HPC = 4          # heads per core
CW = HPC * HD    # 256 channel columns per core
CH = HD + 1      # head channels + ones column
G = CW // P      # 2 channel groups of 128
KC = DIM // P    # 8 contraction chunks for projections
NT = 512         # matmul moving-dim tile

_cache = {}


def _build(L, M):
    LT = L // NT              # q strips
    MT = M // NT              # k strips
    MG = M // P               # m chunks for attention
    L5 = L // NT              # attention l-strips per head pair
    LC = L // P               # out-proj l chunks
    JT = DIM // NT            # out-proj j tiles

    nc = bacc.Bacc()
    # strip-major host-packed layouts: every DMA is contiguous per partition
    qTs = nc.declare_dram_parameter("qTs", [LT, P, KC, NT], bf16, isOutput=False)
    kTs = nc.declare_dram_parameter("kTs", [MT, P, KC, NT], bf16, isOutput=False)
    wq = nc.declare_dram_parameter("wq", [P, KC, CW], bf16, isOutput=False)
    wk = nc.declare_dram_parameter("wk", [P, KC, CW], bf16, isOutput=False)
    wo = nc.declare_dram_parameter("wo", [P, G, DIM], bf16, isOutput=False)
    out = nc.declare_dram_parameter("out", [L, DIM], f32, isOutput=True)
    den_dram = nc.dram_tensor("den_scratch", [HPC, L], f32)
    rden_dram = nc.dram_tensor("rden_scratch", [HPC, L], f32)

    with tile.TileContext(nc) as tc:
        with (
            tc.tile_pool(name="singles", bufs=1) as singles,
            tc.tile_pool(name="io", bufs=2) as io,
            tc.tile_pool(name="es", bufs=4) as es_pool,
            tc.tile_pool(name="opool", bufs=3) as opool,
            tc.tile_pool(name="dstp", bufs=2) as dstp,
        ):
            wq_sb = singles.tile([P, KC, CW], bf16)
            wk_sb = singles.tile([P, KC, CW], bf16)
            wo_sb = singles.tile([P, G, DIM], bf16)
            kTr = singles.tile([P, MT, KC, NT], bf16)   # resident k^T
            qTr = singles.tile([P, LT, KC, NT], bf16)   # resident q^T

            qhT = singles.tile([P, G, L], bf16)
            khT = singles.tile([P, G, M], bf16)
            # per-head stride padded to 128 elems: XBAR transpose dst must be
            # 256B-aligned; cols 0:64 = Kh^T, col 64 = ones, 65:128 dead
            khp = singles.tile([P, MG, HPC, P], bf16)
            xu = singles.tile([P, G, L], bf16)
            rdbc = singles.tile([P, G, L], f32)

            ones_sb = singles.tile([P, 1], f32)
            nc.vector.memset(ones_sb, 1.0)
            for mg in range(MG):
                nc.vector.tensor_copy(khp[:, mg, :, HD:CH],
                                      ones_sb[:, None, :].to_broadcast([P, HPC, 1]))

            # ---- DMA queue plan (2 HW queues: sync + scalar) -------------
            # sync:   wk, kTs0..3, [out stores later] — the k-projection path
            # scalar: wq, qTs0..3, wo — the q path + late-needed wo
            nc.sync.dma_start(wk_sb, wk[:, :, :])
            for mt in range(MT):
                nc.sync.dma_start(kTr[:, mt], kTs[mt])
            nc.scalar.dma_start(wq_sb, wq[:, :, :])
            for lt in range(LT):
                nc.scalar.dma_start(qTr[:, lt], qTs[lt])
            nc.scalar.dma_start(wo_sb, wo[:, :, :])

            from concourse.masks import make_identity
            ident = singles.tile([P, P], bf16)
            make_identity(nc, ident)

            with (
                tc.tile_pool(name="pw", bufs=1, space="PSUM") as pw,
                tc.tile_pool(name="psS", bufs=2, space="PSUM") as psS,
                tc.tile_pool(name="psX", bufs=2, space="PSUM") as psX,
            ):
                def _proj_a(srcr, w_sb, tt, g, st):
                    ps = pw.tile([P, NT], f32, tag="pw")
                    for kc in range(KC // 2):
                        nc.tensor.matmul(ps, lhsT=w_sb[:, kc, ts(g, P)],
                                         rhs=srcr[:, tt, kc],
                                         start=(kc == 0), stop=False)
                    st["ps"] = ps

                def _proj_b(dst, srcr, w_sb, tt, g, st):
                    ps = st["ps"]
                    for kc in range(KC // 2, KC):
                        nc.tensor.matmul(ps, lhsT=w_sb[:, kc, ts(g, P)],
                                         rhs=srcr[:, tt, kc],
                                         start=False, stop=(kc == KC - 1))
                    nc.vector.tensor_copy(dst[:, g, ts(tt, NT)], ps)

                def qproj(lt, g):
                    st = {}
                    _proj_a(qTr, wq_sb, lt, g, st)
                    _proj_b(qhT, qTr, wq_sb, lt, g, st)

                def ktrans(mc_lo, mc_hi, g):
                    # k^T chunk -> head-major khp rows via PE transpose
                    for mc in range(mc_lo, mc_hi):
                        tr = pw.tile([P, P], bf16, tag="pt")
                        nc.tensor.transpose(tr, khT[:, g, ts(mc, P)], ident)
                        for hh in range(2):
                            nc.vector.tensor_copy(khp[:, mc, g * 2 + hh, 0:HD],
                                                  tr[:, ts(hh, HD)])

                # ---- prologue: kproj g0 + qproj lt0 g0, PE gap-free ------
                for mt in range(MT):
                    st = {}
                    _proj_a(kTr, wk_sb, mt, 0, st)
                    _proj_b(khT, kTr, wk_sb, mt, 0, st)
                    ktrans(4 * mt, 4 * mt + 4, 0)
                qproj(0, 0)

                # ---- filler actions for attention g0 PE slack ------------
                fillers = []

                def kproj_g1(mt):
                    st = {}
                    fillers.append(lambda: _proj_a(kTr, wk_sb, mt, 1, st))
                    fillers.append(lambda: _proj_b(khT, kTr, wk_sb, mt, 1, st))

                def qproj_f(lt, g):
                    st = {}
                    fillers.append(lambda: _proj_a(qTr, wq_sb, lt, g, st))
                    fillers.append(lambda: _proj_b(qhT, qTr, wq_sb, lt, g, st))

                qproj_f(1, 0)
                kproj_g1(0)
                qproj_f(2, 0)
                kproj_g1(1)
                qproj_f(3, 0)
                kproj_g1(2)
                kproj_g1(3)
                for mc_lo in range(0, MG, 4):
                    fillers.append(lambda a=mc_lo: ktrans(a, a + 4, 1))
                for lt in range(LT):
                    qproj_f(lt, 1)

                # out-projection tile (2 matmuls + copy + store)
                def out_tile(lc, jt, eng):
                    po = pw.tile([P, NT], f32, tag="pw")
                    for cc in range(G):
                        nc.tensor.matmul(po, lhsT=xu[:, cc, ts(lc, P)],
                                         rhs=wo_sb[:, cc, ts(jt, NT)],
                                         start=(cc == 0), stop=(cc == G - 1))
                    ot = opool.tile([P, NT], f32, tag="ot")
                    nc.vector.tensor_copy(ot, po)
                    eng.dma_start(out[ts(lc, P), ts(jt, NT)], ot)

                pending = []  # out-proj (lc, jt) pairs, filled per finished g1 strip

                # ---- attention ------------------------------------------
                for g in range(G):
                    hA, hB = 2 * g, 2 * g + 1
                    if g == 1:
                        while fillers:
                            fillers.pop(0)()
                    for l5 in range(L5):
                        lsl = ts(l5, NT)

                        def emit_sp(mc, g=g, lsl=lsl):
                            sps = psS.tile([P, 2 * NT], f32, tag="s")
                            nc.tensor.matmul(sps[:, 0:NT],
                                             lhsT=khT[0:HD, g, ts(mc, P)],
                                             rhs=qhT[0:HD, g, lsl],
                                             start=True, stop=True)
                            nc.tensor.matmul(sps[:, NT:2 * NT],
                                             lhsT=khT[HD:P, g, ts(mc, P)],
                                             rhs=qhT[HD:P, g, lsl],
                                             start=True, stop=True)
                            return sps

                        xpsA = psX.tile([CH, NT], f32, tag="x")
                        xpsB = psX.tile([CH, NT], f32, tag="x")
                        sq = [emit_sp(0)]
                        if MG > 1:
                            sq.append(emit_sp(1))
                        for mc in range(MG):
                            if mc + 2 < MG:
                                sq.append(emit_sp(mc + 2))
                            es = es_pool.tile([P, 2 * NT], bf16, tag="es")
                            nc.scalar.activation(es, sq.pop(0), Exp, scale=0.125)
                            nc.tensor.matmul(xpsA, lhsT=khp[:, mc, hA, 0:CH],
                                             rhs=es[:, 0:NT],
                                             start=(mc == 0), stop=(mc == MG - 1))
                            nc.tensor.matmul(xpsB, lhsT=khp[:, mc, hB, 0:CH],
                                             rhs=es[:, NT:2 * NT],
                                             start=(mc == 0), stop=(mc == MG - 1))
                            if mc % 2 == 1:
                                if g == 0 and fillers:
                                    fillers.pop(0)()
                                elif g == 1 and pending:
                                    lc, jt = pending.pop(0)
                                    out_tile(lc, jt, nc.sync)

                        for hh, xps in ((0, xpsA), (1, xpsB)):
                            h = 2 * g + hh
                            pb = hh * HD
                            nc.vector.tensor_copy(xu[pb:pb + HD, g, lsl], xps[0:HD])
                            dstg = dstp.tile([1, NT], f32, tag="dst")
                            nc.vector.tensor_copy(dstg, xps[HD:CH])
                            nc.gpsimd.dma_start(den_dram[h:h + 1, lsl], dstg)
                            dsp_t = io.tile([P, NT // P], f32, tag="dsp")
                            nc.gpsimd.dma_start(
                                dsp_t, den_dram[h, lsl].rearrange("(p f) -> p f", p=P))
                            nc.vector.reciprocal(dsp_t, dsp_t)
                            nc.gpsimd.dma_start(
                                rden_dram[h, lsl].rearrange("(p f) -> p f", p=P), dsp_t)
                            nc.gpsimd.dma_start(
                                rdbc[ts(hh, HD), g, lsl],
                                rden_dram[h:h + 1, lsl].to_broadcast([HD, NT]))
                            nc.vector.tensor_mul(xu[pb:pb + HD, g, lsl],
                                                 xu[pb:pb + HD, g, lsl],
                                                 rdbc[ts(hh, HD), g, lsl])

                        if g == 1:
                            # out-proj for this strip, popped during the next
                            for lci in range(NT // P):
                                for jt in range(JT):
                                    pending.append((l5 * (NT // P) + lci, jt))

                # tail: drain remaining out tiles, alternating DMA queues
                # (the scalar queue is free once the last exp has issued)
                for i, (lc, jt) in enumerate(pending):
                    out_tile(lc, jt, nc.sync if i % 2 == 0 else nc.scalar)

    nc.finalize()
    return nc


def _get_nc(L, M):
    key = (L, M)
    if key not in _cache:
        _cache[key] = _build(L, M)
    return _cache[key]


# head-major channel permutation: new channel c = h*64+d <- original column d*16+h
_PERM = np.array([(c % HD) * NH + c // HD for c in range(DIM)])

last_exec_time_ns = None
last_results = None


def kernel(q, k, v, Wq, Wk, Wv, Wo):  # noqa: ARG001 - v/Wv dead in reference
    global last_exec_time_ns, last_results
    q = np.asarray(q, np.float32)
    k = np.asarray(k, np.float32)
    Wq = np.asarray(Wq, np.float32)
    Wk = np.asarray(Wk, np.float32)
    Wo = np.asarray(Wo, np.float32)
    B, L, _ = q.shape
    M = k.shape[1]
    LT, MT = L // NT, M // NT

    import ml_dtypes
    bf = ml_dtypes.bfloat16
    Wq_p = Wq[_PERM]            # (1024, 1024) head-major rows
    Wk_p = Wk[_PERM]
    WoT_p = Wo[:, _PERM].T      # (1024 c, 1024 j)

    def pack_strips(x):         # x: (S, DIM) -> [S/NT, P, KC, NT]
        S = x.shape[0]
        xt = x.T.reshape(KC, P, S // NT, NT)
        return np.ascontiguousarray(xt.transpose(2, 1, 0, 3)).astype(bf)

    def pack_w(Whg):            # (CW, DIM) -> [P, KC, CW]
        wt = Whg.T.reshape(KC, P, CW)
        return np.ascontiguousarray(wt.transpose(1, 0, 2)).astype(bf)

    def pack_wo(WoThg):         # (CW, DIM) -> [P, G, DIM]
        wt = WoThg.reshape(G, P, DIM)
        return np.ascontiguousarray(wt.transpose(1, 0, 2)).astype(bf)

    qTs = [pack_strips(q[b]) for b in range(B)]
    kTs = [pack_strips(k[b]) for b in range(B)]
    wq_l = [pack_w(Wq_p[hg * CW:(hg + 1) * CW, :]) for hg in range(4)]
    wk_l = [pack_w(Wk_p[hg * CW:(hg + 1) * CW, :]) for hg in range(4)]
    wo_l = [pack_wo(WoT_p[hg * CW:(hg + 1) * CW, :]) for hg in range(4)]

    in_maps = []
    for core in range(8):
        b, hg = divmod(core, 4)
        in_maps.append({"qTs": qTs[b], "kTs": kTs[b], "wq": wq_l[hg],
                        "wk": wk_l[hg], "wo": wo_l[hg]})

    nc = _get_nc(L, M)
    trace = bool(int(os.environ.get("MHA_TRACE", "0")))
    res = run_bass_kernel_spmd(nc, in_maps, core_ids=list(range(8)), trace=trace)
    last_results = res
    last_exec_time_ns = res.exec_time_ns

    out = np.zeros((B, L, DIM), np.float32)
    for core in range(8):
        b = core // 4
        out[b] += res.results[core]["out"]
    return out
